# revision 1
# baseline (speedup 1.0000x reference)
"""Causal multi-head attention block (B=2, S=2048, M=1024, H=16, D=64) for 8
Trainium2 NeuronCores.

Sharding: tensor-parallel over heads (2 heads per core). Each core computes
QKV for its heads from the full x, runs causal attention, then an AllToAll
re-shards z so every core computes its 512-row slice of the output
projection against the full W_proj. Matmuls run in float32r (full PE rate,
~1e-4 relative error). Phase-1 (QKV) and phase-2 (attention) emission is
interleaved so ACT exp work overlaps PE matmul work.

Self-contained: hardcodes all shapes; host-side numpy only shards/transposes
inputs and concatenates outputs.
"""

import numpy as np

import concourse.bass as bass
import concourse.bacc as bacc
import concourse.mybir as mybir
import concourse.tile as tile
from concourse.bass_utils import run_bass_kernel_spmd

B, S, M, H, D = 2, 2048, 1024, 16, 64
NC = 8
R = B * S                  # 4096 rows
HPC = H // NC              # 2 heads per core
MC = HPC * D               # 128 m-columns per core
P = 128
RB = 512                   # phase-1 row block
QB = 512                   # phase-2 query block
NRB = R // RB              # 8
NQB = S // QB              # 4 query blocks per batch
NMT = M // P               # 8 m-tiles
NVT = R // P               # 32 V row tiles
ROWS_PC = R // NC          # 512 output rows per core

f32 = mybir.dt.float32
f32r = mybir.dt.float32r
AF = mybir.ActivationFunctionType
ALU = mybir.AluOpType

_BUILD_CACHE = {}

TUNE = {"acc_bufs": 2, "tp_own": False, "tp_bufs": 2, "st2_bufs": 2,
        "ex_bufs": 4, "xp_bufs": 3, "wp_late": True, "b1_st2": 3,
        "di3_narrow": False, "no_coll": False, "early_free": True}


def build_nc(with_bias=False, for_sim=False, phases=3, repeat=1):
    key = ("nc", with_bias, for_sim, phases, repeat,
           tuple(sorted(TUNE.items())))
    if key in _BUILD_CACHE:
        return _BUILD_CACHE[key]
    nc = bacc.Bacc("TRN2", target_bir_lowering=False, debug=False,
                   num_devices=1 if for_sim else NC)

    xT = nc.dram_tensor("xT", [M, R], f32r, kind="ExternalInput").ap()
    wq = nc.dram_tensor("wq", [M, MC], f32r, kind="ExternalInput").ap()
    wk = nc.dram_tensor("wk", [M, MC], f32r, kind="ExternalInput").ap()
    wv = nc.dram_tensor("wv", [M, MC], f32r, kind="ExternalInput").ap()
    bqkv = nc.dram_tensor("bqkv", [P, 3], f32, kind="ExternalInput").ap()
    wp = nc.dram_tensor("wp", [M, M], f32r, kind="ExternalInput").ap()
    tri1 = nc.dram_tensor("tri1", [P, 128], f32r, kind="ExternalInput").ap()
    tri2 = nc.dram_tensor("tri2", [P, 256], f32r, kind="ExternalInput").ap()
    ident_d = nc.dram_tensor("ident_d", [P, P], f32r, kind="ExternalInput").ap()
    c65 = nc.dram_tensor("c65", [P, 65], f32r, kind="ExternalInput").ap()

    out = nc.dram_tensor("out", [ROWS_PC, M], f32, kind="ExternalOutput").ap()

    with tile.TileContext(nc) as tc:
        with (
            tc.tile_pool(name="cb", bufs=1) as cb,        # constants / persistents
            tc.tile_pool(name="dram", bufs=1, space="DRAM") as dram,
        ):
            # ---- constants ----
            wq_sb = cb.tile([P, NMT, MC], f32r)
            wk_sb = cb.tile([P, NMT, MC], f32r)
            wv_sb = cb.tile([P, NMT, MC], f32r)
            nc.sync.dma_start(wq_sb[:], wq.rearrange("(mt p) d -> p mt d", p=P))
            nc.sync.dma_start(wk_sb[:], wk.rearrange("(mt p) d -> p mt d", p=P))
            nc.sync.dma_start(wv_sb[:], wv.rearrange("(mt p) d -> p mt d", p=P))
            bias_sb = cb.tile([P, 3], f32)
            nc.sync.dma_start(bias_sb[:], bqkv[:])
            tri1_sb = cb.tile([P, 128], f32r)
            tri2_sb = cb.tile([P, 256], f32r)
            ident = cb.tile([P, P], f32r)
            c65_sb = cb.tile([P, 65], f32r)
            nc.sync.dma_start(tri1_sb[:], tri1[:])
            nc.sync.dma_start(tri2_sb[:], tri2[:])
            nc.sync.dma_start(ident[:], ident_d[:])
            nc.sync.dma_start(c65_sb[:], c65[:])

            # ---- persistent activations ----
            QT = cb.tile([P, R], f32r)        # [2h*64, rows], q pre-scaled
            KT = cb.tile([P, R], f32r)
            VA = cb.tile([P, NVT, 65], f32r)   # [V_A | ones]
            VB = cb.tile([P, NVT, P], f32r)    # [ones | 0*63 | V_B]
            ZT = cb.tile([P, R], f32r)

            # phase-3 weights
            wp_sb = cb.tile([P, NMT, M], f32r)
            if not TUNE["wp_late"]:
                nc.sync.dma_start(wp_sb[:],
                                  wp.rearrange("(mt p) n -> p mt n", p=P))

            a2a_in = dram.tile([M, ROWS_PC], f32)
            a2a_out = dram.tile([M, ROWS_PC], f32)

            def copy_cast(dst, src, which):
                if with_bias:
                    nc.scalar.activation(dst, src, AF.Identity,
                                         bias=bias_sb[:, which:which + 1])
                else:
                    nc.vector.tensor_copy(dst, src)

            if True:
                def emit_ph1(rb, ps1, acc_bufs, tp_bufs, xp, vp):
                    r0 = rb * RB
                    xt = xp.tile([P, NMT, RB], f32r, tag="xt", name="xt")
                    for mt in range(NMT):
                        nc.sync.dma_start(
                            xt[:, mt, :], xT[mt * P:(mt + 1) * P, r0:r0 + RB])
                    for which, (w_sb, dst) in enumerate(
                            ((wq_sb, QT), (wk_sb, KT), (wv_sb, None))):
                        acc = ps1.tile([P, RB], f32, tag="u", name="acc",
                                       bufs=acc_bufs)
                        for mt in range(NMT):
                            nc.tensor.matmul(acc[:], w_sb[:, mt, :], xt[:, mt, :],
                                             start=(mt == 0), stop=(mt == NMT - 1))
                        if dst is not None:
                            copy_cast(dst[:, r0:r0 + RB], acc[:], which)
                        else:
                            vt_sb = vp.tile([P, RB], f32r, tag="vt", name="vt_sb")
                            copy_cast(vt_sb[:], acc[:], which)
                            for k in range(RB // P):
                                t = rb * (RB // P) + k
                                tp = ps1.tile(
                                    [P, P], f32r, name="tp",
                                    tag="tp" if tp_bufs else "u",
                                    bufs=tp_bufs if tp_bufs else acc_bufs)
                                nc.tensor.transpose(
                                    tp[:], vt_sb[:, k * P:(k + 1) * P], ident[:])
                                nc.vector.tensor_copy(VA[:, t, 0:64], tp[:, 0:64])
                                nc.vector.tensor_copy(VA[:, t, 64:65],
                                                      c65_sb[:, 0:1])
                                nc.vector.tensor_copy(VB[:, t, 0:64],
                                                      c65_sb[:, 0:64])
                                nc.vector.tensor_copy(VB[:, t, 64:128],
                                                      tp[:, 64:128])

                def emit_ph2(b, qb, ps2, exp_pool, norm_pool, st2_bufs=None):
                    gr0 = b * S + qb * QB
                    zt_a = ps2.tile([65, QB], f32, tag="zt", bufs=2, name="zt_a")
                    zt_b = ps2.tile([P, QB], f32, tag="zt", bufs=2, name="zt_b")
                    nkj = 4 * qb + 4
                    for t in range(nkj):
                        kj0 = 128 * t
                        di = t - 4 * qb
                        if di < 0:
                            col_off, w = 0, QB
                        elif di <= 1:
                            col_off, w = 128 * di, QB - 128 * di
                        elif di == 3 and TUNE["di3_narrow"]:
                            col_off, w = 384, 128
                        else:
                            col_off, w = 256, 256
                        st2 = ps2.tile([P, 2 * QB], f32, tag="st2",
                                       bufs=st2_bufs or TUNE["st2_bufs"],
                                       name="st2")
                        for h in range(2):
                            hp = slice(64 * h, 64 * h + 64)
                            nc.tensor.matmul(
                                st2[:, h * QB:h * QB + w],
                                KT[hp, b * S + kj0: b * S + kj0 + 128],
                                QT[hp, gr0 + col_off: gr0 + col_off + w],
                                start=True, stop=True)
                        ex = exp_pool.tile([P, 2, QB], f32r, tag="ex", name="ex")
                        st2v = st2.rearrange("p (h q) -> p h q", h=2)
                        nc.scalar.activation(ex[:, :, :w], st2v[:, :, :w], AF.Exp)
                        if di >= 0:
                            if di == 3 and not TUNE["di3_narrow"]:
                                nc.vector.tensor_tensor(
                                    ex[:, :, 0:256], ex[:, :, 0:256],
                                    tri2_sb[:, None, :].to_broadcast([P, 2, 256]),
                                    ALU.mult)
                            else:
                                nc.vector.tensor_tensor(
                                    ex[:, :, 0:128], ex[:, :, 0:128],
                                    tri1_sb[:, None, :].to_broadcast([P, 2, 128]),
                                    ALU.mult)
                        vt_idx = 16 * b + t
                        for h, (zt_x, vx) in enumerate(((zt_a, VA), (zt_b, VB))):
                            nc.tensor.matmul(
                                zt_x[:, col_off:col_off + w], vx[:, vt_idx, :],
                                ex[:, h, :w],
                                start=(t == 0), stop=(t == nkj - 1),
                                skip_group_check=True)
                    # normalize into ZT
                    recip = norm_pool.tile([P, QB], f32, tag="recip", name="recip")
                    nc.vector.reciprocal(recip[64:65, :], zt_a[64:65, :])
                    nc.vector.reciprocal(recip[0:1, :], zt_b[0:1, :])
                    if TUNE["early_free"]:
                        # copy z out of PSUM immediately so the zt slots free
                        # without waiting on the DMA+broadcast chain; the
                        # final multiply is then all-SBUF (DVE 2x mode)
                        zsrc = norm_pool.tile([P, QB], f32r, tag="zc",
                                              name="zc")
                        nc.vector.tensor_copy(zsrc[0:64, :], zt_a[0:64, :])
                        nc.vector.tensor_copy(zsrc[64:128, :], zt_b[64:128, :])
                    else:
                        zsrc = None
                    rowa = norm_pool.tile([1, QB], f32, tag="rowa", name="rowa")
                    nc.sync.dma_start(rowa[:], recip[64:65, :])
                    bca = norm_pool.tile([64, QB], f32, tag="bca", name="bca")
                    bcb = norm_pool.tile([P, QB], f32, tag="bcb", name="bcb")
                    nc.gpsimd.partition_broadcast(bca[:], rowa[:], channels=64)
                    nc.gpsimd.partition_broadcast(bcb[:], recip[0:1, :],
                                                  channels=128)
                    sa = zsrc[0:64, :] if zsrc is not None else zt_a[0:64, :]
                    sb_ = (zsrc[64:128, :] if zsrc is not None
                           else zt_b[64:128, :])
                    nc.vector.tensor_tensor(ZT[0:64, gr0:gr0 + QB],
                                            sa, bca[:], ALU.mult)
                    nc.vector.tensor_tensor(ZT[64:128, gr0:gr0 + QB],
                                            sb_, bcb[64:128, :], ALU.mult)
                    if phases >= 3:
                        j = gr0 // ROWS_PC
                        nc.sync.dma_start(
                            a2a_in[j * P:(j + 1) * P, :],
                            ZT.bitcast(f32)[:, gr0:gr0 + QB])

                def emit_iter(rep):
                    sfx = f"_{rep}"
                    with (
                        tc.tile_pool(name="xp" + sfx,
                                     bufs=TUNE["xp_bufs"]) as xp,
                        tc.tile_pool(name="vp" + sfx, bufs=2) as vp,
                        tc.tile_pool(name="ex" + sfx,
                                     bufs=TUNE["ex_bufs"]) as exp_pool,
                        tc.tile_pool(name="np" + sfx, bufs=2) as norm_pool,
                    ):
                        # rb0-3 with a wide PSUM pool (closes before ps2)
                        with tc.tile_pool(name="ps1a" + sfx, bufs=1,
                                          space="PSUM") as ps1a:
                            for rb in range(4):
                                emit_ph1(rb, ps1a, 4, 2, xp, vp)
                        # interleaved emission
                        with (
                            tc.tile_pool(name="ps1b" + sfx, bufs=1,
                                         space="PSUM") as ps1b,
                            tc.tile_pool(name="ps2" + sfx, bufs=1,
                                         space="PSUM") as ps2,
                        ):
                            for rb, blk in ((4, (0, 0)), (5, (0, 1)),
                                            (6, (0, 2)), (7, (0, 3))):
                                if phases >= 2:
                                    emit_ph2(*blk, ps2, exp_pool, norm_pool)
                                emit_ph1(rb, ps1b, TUNE["acc_bufs"], 0, xp, vp)
                            if TUNE["wp_late"] and rep == 0:
                                for mt in range(NMT):
                                    nc.sync.dma_start(
                                        wp_sb[:, mt, :],
                                        wp[mt * P:(mt + 1) * P, :])
                        # batch-1 blocks: all of phase 1 is done, use a wide
                        # PSUM pool with triple-buffered score tiles
                        with tc.tile_pool(name="ps2w" + sfx, bufs=1,
                                          space="PSUM") as ps2w:
                            if phases >= 2:
                                for qb in range(NQB):
                                    emit_ph2(1, qb, ps2w, exp_pool, norm_pool,
                                             st2_bufs=TUNE["b1_st2"])

                    # ---- all-to-all ----
                    if phases >= 3:
                        if for_sim or TUNE["no_coll"]:
                            nc.sync.dma_start(a2a_out[:], a2a_in[:])
                        else:
                            nc.gpsimd.collective_compute(
                                "AllToAll", ALU.bypass,
                                replica_groups=[list(range(NC))],
                                ins=[a2a_in.opt()], outs=[a2a_out.opt()],
                            )

                    # ---- phase 3: output projection ----
                    with (
                        tc.tile_pool(name="op" + sfx, bufs=2) as out_pool,
                        tc.tile_pool(name="ps3" + sfx, bufs=4,
                                     space="PSUM") as ps3,
                    ):
                        zt_sb = out_pool.tile([P, NMT, ROWS_PC], f32r, bufs=1,
                                              tag="zt_sb", name="zt_sb")
                        if phases >= 3:
                            for mt in range(NMT):
                                nc.sync.dma_start(
                                    zt_sb[:, mt, :],
                                    a2a_out.bitcast(f32r)[mt * P:(mt + 1) * P, :])
                        for rt in range(ROWS_PC // P if phases >= 3 else 0):
                            os_ = out_pool.tile([P, M], f32, tag="os", name="os_")
                            for nh in range(2):
                                acc = ps3.tile([P, 512], f32, tag="o",
                                               name="acc3")
                                for mt in range(NMT):
                                    nc.tensor.matmul(
                                        acc[:], zt_sb[:, mt, rt * P:(rt + 1) * P],
                                        wp_sb[:, mt, nh * 512:(nh + 1) * 512],
                                        start=(mt == 0), stop=(mt == NMT - 1))
                                nc.vector.tensor_copy(
                                    os_[:, nh * 512:(nh + 1) * 512], acc[:])
                            nc.sync.dma_start(out[rt * P:(rt + 1) * P, :], os_[:])

                for rep in range(repeat):
                    emit_iter(rep)

    nc.compile()
    _BUILD_CACHE[key] = nc
    return nc


def prep_inputs(x, W_attn, b_attn, W_proj, b_proj):
    x = np.asarray(x, dtype=np.float32)
    W_attn = np.asarray(W_attn, dtype=np.float32)
    b_attn = np.asarray(b_attn, dtype=np.float32)
    W_proj = np.asarray(W_proj, dtype=np.float32)

    xT = np.ascontiguousarray(x.reshape(R, M).T)
    tri1 = (np.arange(128)[None, :] >= np.arange(128)[:, None]).astype(np.float32)
    tri2 = (np.arange(256)[None, :] >= (np.arange(128) + 128)[:, None]).astype(
        np.float32)
    ident = np.eye(P, dtype=np.float32)
    c65 = np.zeros((P, 65), dtype=np.float32)
    c65[:, 0] = 1.0
    scale = 1.0 / np.sqrt(D)

    in_maps = []
    for c in range(NC):
        cs = slice(MC * c, MC * (c + 1))
        bq = b_attn[0 * M:1 * M][cs] * scale
        bk = b_attn[1 * M:2 * M][cs]
        bv = b_attn[2 * M:3 * M][cs]
        in_maps.append({
            "xT": xT,
            "wq": np.ascontiguousarray(W_attn[:, 0 * M:1 * M][:, cs] * scale),
            "wk": np.ascontiguousarray(W_attn[:, 1 * M:2 * M][:, cs]),
            "wv": np.ascontiguousarray(W_attn[:, 2 * M:3 * M][:, cs]),
            "bqkv": np.ascontiguousarray(np.stack([bq, bk, bv], axis=1)),
            "wp": W_proj,
            "tri1": tri1, "tri2": tri2, "ident_d": ident, "c65": c65,
        })
    return in_maps


def postprocess(results, b_proj):
    out = np.concatenate([results[c]["out"] for c in range(NC)], axis=0)
    out = out + np.asarray(b_proj, dtype=np.float32)[None, :]
    return out.reshape(B, S, M)


def kernel(x, W_attn, b_attn, W_proj, b_proj):
    nc = build_nc(with_bias=bool(np.any(np.asarray(b_attn))))
    in_maps = prep_inputs(x, W_attn, b_attn, W_proj, b_proj)
    res = run_bass_kernel_spmd(nc, in_maps, core_ids=list(range(NC)))
    return postprocess(res.results, b_proj)



# revision 38
# speedup vs baseline: 1.7685x; 1.7685x over previous
"""Causal multi-head attention block (B=2, S=2048, M=1024, H=16, D=64) for 8
Trainium2 NeuronCores.

Sharding: tensor-parallel over heads (2 heads per core). Each core computes
QKV for its heads from the full x (bf16), runs causal attention, then two
AllToAlls (one per batch) re-shard z so every core computes its 512 output
rows (64-row interleaved ownership) against the full W_proj. The first
AllToAll is issued at the batch boundary and hides under batch-1 attention;
only the second is on the critical path. The causal mask is applied by
accumulating a -30000 triangular matrix into the score PSUM via a bf16
matmul, keeping the DVE off the score->exp->AV critical chain, and the AV
matmul for key-tile t-1 is emitted after the score matmul for tile t so the
PE never waits on the ACT engine's exp.

Self-contained: hardcodes all shapes; host-side numpy shards/transposes
inputs and reorders/concatenates outputs.
"""

import numpy as np

import concourse.bass as bass
import concourse.bacc as bacc
import concourse.mybir as mybir
import concourse.tile as tile
from concourse.bass_utils import run_bass_kernel_spmd

B, S, M, H, D = 2, 2048, 1024, 16, 64
NC = 8
R = B * S                  # 4096 rows
HPC = H // NC              # 2 heads per core
MC = HPC * D               # 128 m-columns per core
P = 128
RB = 512                   # phase-1 row block
QB = 512                   # phase-2 query block
NRB = R // RB              # 8
NQB = S // QB              # 4 query blocks per batch
NMT = M // P               # 8 m-tiles
NVT = R // P               # 32 V row tiles
ROWS_PC = R // NC          # 512 output rows per core
NEG = -30000.0

f32 = mybir.dt.float32
bf16 = mybir.dt.bfloat16
AF = mybir.ActivationFunctionType
ALU = mybir.AluOpType

_BUILD_CACHE = {}

TUNE = {"st2_bufs": 2, "b1_st2": 3, "zt_b1": 2, "ex_bufs": 4, "xp_bufs": 3,
        "acc_bufs": 2, "acc_bufs_a": 6, "dma_tp": False, "no_coll": False,
        "wsplit": False, "xt_dmas": 8, "qk_dve": True, "a2a_one": True, "os_split": True, "pe_bc": False}


def build_nc(with_bias=False, for_sim=False, phases=3, repeat=1):
    key = ("nc", with_bias, for_sim, phases, repeat,
           tuple(sorted(TUNE.items())))
    if key in _BUILD_CACHE:
        return _BUILD_CACHE[key]
    nc = bacc.Bacc("TRN2", target_bir_lowering=False, debug=False,
                   num_devices=1 if for_sim else NC)

    xT = nc.dram_tensor("xT", [M, R], bf16, kind="ExternalInput").ap()
    wq = nc.dram_tensor("wq", [M, MC], bf16, kind="ExternalInput").ap()
    wk = nc.dram_tensor("wk", [M, MC], bf16, kind="ExternalInput").ap()
    wv = nc.dram_tensor("wv", [M, MC], bf16, kind="ExternalInput").ap()
    bqkv = nc.dram_tensor("bqkv", [P, 3], f32, kind="ExternalInput").ap()
    wp = nc.dram_tensor("wp", [M, M], bf16, kind="ExternalInput").ap()
    maskT = nc.dram_tensor("maskT", [P, P], bf16, kind="ExternalInput").ap()
    ident_d = nc.dram_tensor("ident_d", [P, P], bf16, kind="ExternalInput").ap()

    out = nc.dram_tensor("out", [ROWS_PC, M], f32, kind="ExternalOutput").ap()

    local_coll = for_sim or TUNE["no_coll"]

    with tile.TileContext(nc) as tc:
        with (
            tc.tile_pool(name="cb", bufs=1) as cb,        # constants / persistents
            tc.tile_pool(name="dram", bufs=1, space="DRAM") as dram,
        ):
            # ---- constants ----
            wq_sb = cb.tile([P, NMT, MC], bf16)
            wk_sb = cb.tile([P, NMT, MC], bf16)
            wv_sb = cb.tile([P, NMT, MC], bf16)
            if TUNE["wsplit"]:
                for mt in range(NMT):
                    nc.sync.dma_start(wq_sb[:, mt, :],
                                      wq[mt * P:(mt + 1) * P, :])
            else:
                nc.sync.dma_start(
                    wq_sb[:], wq.rearrange("(mt p) d -> p mt d", p=P))
            for w_sb, w_d in ((wk_sb, wk), (wv_sb, wv)):
                nc.gpsimd.dma_start(
                    w_sb[:], w_d.rearrange("(mt p) d -> p mt d", p=P))
            bias_sb = cb.tile([P, 3], f32)
            if with_bias:
                nc.sync.dma_start(bias_sb[:], bqkv[:])
            maskT_sb = cb.tile([P, P], bf16)
            ident = cb.tile([P, P], bf16)
            nc.gpsimd.dma_start(maskT_sb[:], maskT[:])
            nc.gpsimd.dma_start(ident[:], ident_d[:])

            # ---- persistent activations ----
            QT = cb.tile([P, R], bf16)        # [2h*64, rows], q pre-scaled
            KT = cb.tile([P, R], bf16)
            VA = cb.tile([P, NVT, 65], bf16)   # [V_A | ones]
            VB = cb.tile([P, NVT, P], bf16)    # [ones | 0*63 | V_B]
            ZT = cb.tile([P, R], bf16)
            zt_sb = cb.tile([P, NMT, ROWS_PC], bf16)   # phase-3 stationary

            nc.gpsimd.memset(VA[:, :, 64:65], 1.0)
            nc.gpsimd.memset(VB[:, :, 0:1], 1.0)
            nc.gpsimd.memset(VB[:, :, 1:64], 0.0)
            ones_sb = cb.tile([P, 64], bf16)
            nc.gpsimd.memset(ones_sb[:], 1.0)

            # phase-3 weights
            wp_sb = cb.tile([P, NMT, M], bf16)

            a2a_in = [dram.tile([M, 256], bf16, name=f"a2a_in{h}")
                      for h in range(2)]
            a2a_out = [dram.tile([M, 256], bf16, name=f"a2a_out{h}")
                       for h in range(2)]

            def copy_cast(dst, src, which):
                if with_bias:
                    nc.scalar.activation(dst, src, AF.Identity,
                                         bias=bias_sb[:, which:which + 1])
                elif which == 2 or not TUNE["qk_dve"]:
                    nc.scalar.activation(dst, src, AF.Copy)
                else:
                    nc.vector.tensor_copy(dst, src)

            def emit_ph1(rb, ps1, acc_bufs, xp, vp):
                r0 = rb * RB
                xt = xp.tile([P, NMT, RB], bf16, tag="xt", name="xt")
                nd = TUNE["xt_dmas"]
                mtc = NMT // nd
                for i in range(nd):
                    eng = nc.scalar if (rb == 0 and i % 2 == 1) else nc.sync
                    eng.dma_start(
                        xt[:, i * mtc:(i + 1) * mtc, :],
                        xT[i * mtc * P:(i + 1) * mtc * P,
                           r0:r0 + RB].rearrange("(mt p) r -> p mt r", p=P))
                for which, (w_sb, dst) in enumerate(
                        ((wq_sb, QT), (wk_sb, KT), (wv_sb, None))):
                    acc = ps1.tile([P, RB], f32, tag="u", name="acc",
                                   bufs=acc_bufs)
                    for mt in range(NMT):
                        nc.tensor.matmul(acc[:], w_sb[:, mt, :], xt[:, mt, :],
                                         start=(mt == 0), stop=(mt == NMT - 1))
                    if dst is not None:
                        copy_cast(dst[:, r0:r0 + RB], acc[:], which)
                    else:
                        vt_sb = vp.tile([P, RB], bf16, tag="vt", name="vt_sb")
                        copy_cast(vt_sb[:], acc[:], which)
                        for k in range(RB // P):
                            t = rb * (RB // P) + k
                            if TUNE["dma_tp"]:
                                nc.sync.dma_start(
                                    VA[:, t, 0:64],
                                    vt_sb[0:64, k * P:(k + 1) * P],
                                    transpose=True)
                                nc.sync.dma_start(
                                    VB[:, t, 64:128],
                                    vt_sb[64:128, k * P:(k + 1) * P],
                                    transpose=True)
                            else:
                                tp = ps1.tile([P, P], bf16, name="tp", tag="u",
                                              bufs=acc_bufs)
                                nc.tensor.transpose(
                                    tp[:], vt_sb[:, k * P:(k + 1) * P],
                                    ident[:])
                                nc.vector.tensor_copy(VA[:, t, 0:64],
                                                      tp[:, 0:64])
                                nc.vector.tensor_copy(VB[:, t, 64:128],
                                                      tp[:, 64:128])

            def emit_ph2(b, qb, ps2, exp_pool, norm_pool, st2_bufs,
                         zt_bufs=2, prev_norm=None):
                gr0 = b * S + qb * QB
                zt_a = ps2.tile([65, QB], f32, tag="zt", bufs=zt_bufs,
                                name="zt_a")
                zt_b = ps2.tile([P, QB], f32, tag="zt", bufs=zt_bufs,
                                name="zt_b")
                nkj = 4 * qb + 4

                def emit_av(t, ex, w, col_off):
                    vt_idx = 16 * b + t
                    for h, (zt_x, vx) in enumerate(((zt_a, VA), (zt_b, VB))):
                        nc.tensor.matmul(
                            zt_x[:, col_off:col_off + w], vx[:, vt_idx, :],
                            ex[:, h, :w],
                            start=(t == 0), stop=(t == nkj - 1),
                            skip_group_check=True)

                pend = None
                for t in range(nkj):
                    kj0 = 128 * t
                    di = t - 4 * qb
                    if di < 0:
                        col_off, w = 0, QB
                    elif di == 3:
                        col_off, w = 384, 128
                    else:
                        col_off, w = 128 * di, QB - 128 * di
                    st2 = ps2.tile([P, 2 * QB], f32, tag="st2",
                                   bufs=st2_bufs, name="st2")
                    for h in range(2):
                        hp = slice(64 * h, 64 * h + 64)
                        nc.tensor.matmul(
                            st2[:, h * QB:h * QB + w],
                            KT[hp, b * S + kj0: b * S + kj0 + 128],
                            QT[hp, gr0 + col_off: gr0 + col_off + w],
                            start=True, stop=(di < 0), skip_group_check=True)
                    if di >= 0:
                        for h in range(2):
                            nc.tensor.matmul(
                                st2[:, h * QB:h * QB + 128],
                                maskT_sb[:], ident[:],
                                start=False, stop=True, skip_group_check=True)
                    ex = exp_pool.tile([P, 2, QB], bf16, tag="ex", name="ex")
                    st2v = st2.rearrange("p (h q) -> p h q", h=2)
                    nc.scalar.activation(ex[:, :, :w], st2v[:, :, :w], AF.Exp)
                    if pend is not None:
                        emit_av(*pend)
                    if t == min(1, nkj - 1) and prev_norm is not None:
                        prev_norm()
                    pend = (t, ex, w, col_off)
                emit_av(*pend)

                if TUNE["pe_bc"]:
                    recip = norm_pool.tile([P, QB], bf16, tag="recip",
                                           name="recip")
                    with nc.allow_low_precision(reason="bf16 softmax denom"):
                        nc.vector.reciprocal(recip[64:65, :], zt_a[64:65, :])
                        nc.vector.reciprocal(recip[0:1, :], zt_b[0:1, :])
                def do_norm():
                    if TUNE["pe_bc"]:
                        bcab = ps2.tile([P, QB], f32, tag="st2",
                                        bufs=st2_bufs, name="bcab")
                        nc.tensor.matmul(bcab[0:64, :], ones_sb[64:65, :],
                                         recip[64:65, :], start=True,
                                         stop=True)
                        nc.tensor.matmul(bcab[64:128, :], ones_sb[0:1, :],
                                         recip[0:1, :], start=True, stop=True)
                        nc.vector.tensor_tensor(ZT[0:64, gr0:gr0 + QB],
                                                zt_a[0:64, :], bcab[0:64, :],
                                                ALU.mult)
                        nc.vector.tensor_tensor(ZT[64:128, gr0:gr0 + QB],
                                                zt_b[64:128, :],
                                                bcab[64:128, :], ALU.mult)
                    else:
                        recip2 = norm_pool.tile([P, QB], f32, tag="recip",
                                                name="recip")
                        nc.vector.reciprocal(recip2[64:65, :], zt_a[64:65, :])
                        nc.vector.reciprocal(recip2[0:1, :], zt_b[0:1, :])
                        zsrc = norm_pool.tile([P, QB], f32, tag="zc",
                                              name="zc")
                        nc.vector.tensor_copy(zsrc[0:64, :], zt_a[0:64, :])
                        nc.vector.tensor_copy(zsrc[64:128, :],
                                              zt_b[64:128, :])
                        rowa = norm_pool.tile([1, QB], f32, tag="rowa",
                                              name="rowa")
                        nc.sync.dma_start(rowa[:], recip2[64:65, :])
                        bca = norm_pool.tile([64, QB], f32, tag="bca",
                                             name="bca")
                        bcb = norm_pool.tile([P, QB], f32, tag="bcb",
                                             name="bcb")
                        nc.gpsimd.partition_broadcast(bca[:], rowa[:],
                                                      channels=64)
                        nc.gpsimd.partition_broadcast(bcb[:], recip2[0:1, :],
                                                      channels=128)
                        nc.vector.tensor_tensor(ZT[0:64, gr0:gr0 + QB],
                                                zsrc[0:64, :], bca[:],
                                                ALU.mult)
                        nc.vector.tensor_tensor(ZT[64:128, gr0:gr0 + QB],
                                                zsrc[64:128, :],
                                                bcb[64:128, :], ALU.mult)
                    if phases >= 3:
                        dst = a2a_in[b][:, qb * 64:(qb + 1) * 64]
                        if TUNE["a2a_one"]:
                            nc.sync.dma_start(
                                dst.rearrange("(c p) w -> p c w", c=NC),
                                ZT[:, gr0:gr0 + QB].rearrange(
                                    "p (c w) -> p c w", c=NC))
                        else:
                            for c in range(NC):
                                nc.sync.dma_start(
                                    dst[c * P:(c + 1) * P, :],
                                    ZT[:, gr0 + c * 64: gr0 + (c + 1) * 64])
                return do_norm

            def emit_coll(h):
                if local_coll:
                    nc.sync.dma_start(a2a_out[h][:], a2a_in[h][:])
                else:
                    nc.gpsimd.collective_compute(
                        "AllToAll", ALU.bypass,
                        replica_groups=[list(range(NC))],
                        ins=[a2a_in[h].opt()], outs=[a2a_out[h].opt()],
                    )
                for mt in range(NMT):
                    nc.sync.dma_start(
                        zt_sb[:, mt, h * 256:(h + 1) * 256],
                        a2a_out[h][mt * P:(mt + 1) * P, :])

            def emit_ph3(rt, out_pool, ps3, tag="o", bufs=4):
                os_ = out_pool.tile([P, M], f32, tag="os", name="os_")
                for nh in range(2):
                    acc = ps3.tile([P, 512], f32, tag=tag, name="acc3",
                                   bufs=bufs)
                    for mt in range(NMT):
                        nc.tensor.matmul(
                            acc[:], zt_sb[:, mt, rt * P:(rt + 1) * P],
                            wp_sb[:, mt, nh * 512:(nh + 1) * 512],
                            start=(mt == 0), stop=(mt == NMT - 1))
                    nc.scalar.activation(os_[:, nh * 512:(nh + 1) * 512],
                                         acc[:], AF.Copy)
                    if TUNE["os_split"]:
                        nc.sync.dma_start(
                            out[rt * P:(rt + 1) * P,
                                nh * 512:(nh + 1) * 512],
                            os_[:, nh * 512:(nh + 1) * 512])
                if not TUNE["os_split"]:
                    nc.sync.dma_start(out[rt * P:(rt + 1) * P, :], os_[:])

            def emit_iter(rep):
                sfx = f"_{rep}"
                with (
                    tc.tile_pool(name="xp" + sfx,
                                 bufs=TUNE["xp_bufs"]) as xp,
                    tc.tile_pool(name="vp" + sfx, bufs=2) as vp,
                    tc.tile_pool(name="ex" + sfx,
                                 bufs=TUNE["ex_bufs"]) as exp_pool,
                    tc.tile_pool(name="np" + sfx, bufs=2) as norm_pool,
                ):
                    # rb0-3 with a wide PSUM pool (closes before ps2)
                    with tc.tile_pool(name="ps1a" + sfx, bufs=1,
                                      space="PSUM") as ps1a:
                        for rb in range(4):
                            emit_ph1(rb, ps1a, TUNE["acc_bufs_a"], xp, vp)
                    # interleaved emission: ph2 b0 + ph1 rb4-7
                    with (
                        tc.tile_pool(name="ps1b" + sfx, bufs=1,
                                     space="PSUM") as ps1b,
                        tc.tile_pool(name="ps2" + sfx, bufs=1,
                                     space="PSUM") as ps2,
                    ):
                        pn = None
                        for rb, qb in ((4, 0), (5, 1), (6, 2), (7, 3)):
                            if phases >= 2:
                                pn = emit_ph2(0, qb, ps2, exp_pool,
                                              norm_pool, TUNE["st2_bufs"],
                                              prev_norm=pn)
                            emit_ph1(rb, ps1b, TUNE["acc_bufs"], xp, vp)
                            if rep == 0:
                                for mt in (2 * qb, 2 * qb + 1):
                                    nc.gpsimd.dma_start(
                                        wp_sb[:, mt, :],
                                        wp[mt * P:(mt + 1) * P, :])
                        if pn is not None:
                            pn()
                    # first collective: batch-0 z, hides under batch-1 work
                    if phases >= 3:
                        emit_coll(0)
                    # batch-1 blocks: all of phase 1 is done, wide PSUM pool
                    with tc.tile_pool(name="op" + sfx, bufs=2) as out_pool:
                        with tc.tile_pool(name="ps2w" + sfx, bufs=1,
                                          space="PSUM") as ps2w:
                            if phases >= 2:
                                pn = None
                                for qb in range(NQB):
                                    pn = emit_ph2(1, qb, ps2w, exp_pool,
                                                  norm_pool, TUNE["b1_st2"],
                                                  zt_bufs=TUNE["zt_b1"],
                                                  prev_norm=pn)
                                if pn is not None:
                                    pn()
                            if phases >= 3:
                                emit_coll(1)
                                for rt in (0, 1):
                                    emit_ph3(rt, out_pool, ps2w, tag="st2",
                                             bufs=TUNE["b1_st2"])
                        if phases >= 3:
                            with tc.tile_pool(name="ps3" + sfx, bufs=1,
                                              space="PSUM") as ps3:
                                for rt in (2, 3):
                                    emit_ph3(rt, out_pool, ps3)

            for rep in range(repeat):
                emit_iter(rep)

    nc.compile()
    _BUILD_CACHE[key] = nc
    return nc


def prep_inputs(x, W_attn, b_attn, W_proj, b_proj):
    x = np.asarray(x, dtype=np.float32)
    W_attn = np.asarray(W_attn, dtype=np.float32)
    b_attn = np.asarray(b_attn, dtype=np.float32)
    W_proj = np.asarray(W_proj, dtype=np.float32)
    nbf = mybir.dt.np(bf16)

    xT = np.ascontiguousarray(x.reshape(R, M).T).astype(nbf)
    jj = np.arange(P)[None, :]
    pp = np.arange(P)[:, None]
    madd = np.where(jj >= pp, 0.0, NEG).astype(np.float32)   # [key p, q j]
    maskT = np.ascontiguousarray(madd.T).astype(nbf)
    ident = np.eye(P, dtype=np.float32).astype(nbf)
    scale = 1.0 / np.sqrt(D)

    in_maps = []
    for c in range(NC):
        cs = slice(MC * c, MC * (c + 1))
        bq = b_attn[0 * M:1 * M][cs] * scale
        bk = b_attn[1 * M:2 * M][cs]
        bv = b_attn[2 * M:3 * M][cs]
        in_maps.append({
            "xT": xT,
            "wq": np.ascontiguousarray(
                W_attn[:, 0 * M:1 * M][:, cs] * scale).astype(nbf),
            "wk": np.ascontiguousarray(W_attn[:, 1 * M:2 * M][:, cs]).astype(nbf),
            "wv": np.ascontiguousarray(W_attn[:, 2 * M:3 * M][:, cs]).astype(nbf),
            "bqkv": np.ascontiguousarray(np.stack([bq, bk, bv], axis=1)),
            "wp": W_proj.astype(nbf),
            "maskT": maskT, "ident_d": ident,
        })
    return in_maps


# local row r on core c -> global row: half = r//256, j = half*4 + (r%256)//64
# (query block index), global = j*512 + c*64 + r%64
def _row_perm():
    perm = np.empty(NC * ROWS_PC, dtype=np.int64)
    for c in range(NC):
        r = np.arange(ROWS_PC)
        j = (r // 256) * 4 + (r % 256) // 64
        g = j * 512 + c * 64 + (r % 64)
        perm[c * ROWS_PC + r] = g
    return perm


_PERM = _row_perm()


def postprocess(results, b_proj):
    out = np.concatenate([results[c]["out"] for c in range(NC)], axis=0)
    full = np.empty_like(out)
    full[_PERM] = out
    full = full + np.asarray(b_proj, dtype=np.float32)[None, :]
    return full.reshape(B, S, M)


def kernel(x, W_attn, b_attn, W_proj, b_proj):
    nc = build_nc(with_bias=bool(np.any(np.asarray(b_attn))))
    in_maps = prep_inputs(x, W_attn, b_attn, W_proj, b_proj)
    res = run_bass_kernel_spmd(nc, in_maps, core_ids=list(range(NC)))
    return postprocess(res.results, b_proj)
